# revision 12
# baseline (speedup 1.0000x reference)
"""MixerBlock kernel for Trainium2 (8 NeuronCores, data-parallel over batch). v4

Reference computation (per batch b of x[B,T,H], B=32, T=H=1024):
  y   = LN1(x)                                    # over H
  u1  = gelu(W1m @ y + tb1 x 1)    W1m = tril*tw1 # temporal mix in [T,H] layout
  x2  = x + W2m @ u1 + tb2 x 1     W2m = tril*tw2
  y2  = LN2(x2)
  v1  = gelu(cw1' @ y2^T + cb1' x 1)              # [H,T] layout, LN2 g/b folded
  out = x2 + (v1^T' @ cw2T) + 1 x cb2             # back in [T,H]

Design (vs the f32r baseline at 634 us):
- bf16 weights, activations AND residual trunk (PSUM accumulation stays f32;
  x is converted to bf16 on the host). Triangular temporal weight tiles are
  host-packed into [128, 36, 128] so ALL weights stay resident in SBUF
  (~6.25 MB loaded once, was 50 MB of per-batch DMA).
- Software pipeline, one stage deep: iteration `it` runs, back-to-back on
  the PE, MM2(it) -> chMM1(it-1) -> MM1(it+1) -> chMM2(it-1). Every
  LayerNorm chain (stats -> rsqrt -> apply -> transpose) therefore has a
  full channel phase (~55 us) of slack before its consumer - the PE never
  waits on the vector/scalar engines.
- The y2 transpose runs on the DMA engines' XBAR (dma_start(transpose=True),
  out[p,c,j] = in[j,128c+p]) instead of 256 PE transpose matmuls.
- Consecutive matmuls always alternate PSUM banks (incl. chMM2, which pairs
  two output chains) - same-bank back-to-back matmuls serialize fill/drain
  (~405 ns vs ~216 ns per N=512 matmul, HW-measured).
- LayerNorm applies run on the scalar engine as Identity activations
  (out = in*rs + (-mean*rs)); identity/square/gelu share one act table so
  the scalar engine never reloads tables (~2.7 us each). LN2 row sums ride
  the MM2 epilogue's accum_out; sumsq comes from a Square pass.
- 1/sqrt(var+eps) via quake-rsqrt + 2 Newton steps on the vector engine.
- gpsimd only does the cb2-fold (gpsimd ops hold the shared DVE/GpSimd SBUF
  port pair for their whole duration, blocking concurrent DVE ops).
- x loads use the Activation HWDGE queue; stores/transposes use the SP
  queue, so batch-(b+1) loads never queue behind batch-b stores.
"""
import os
import numpy as np
import ml_dtypes
from contextlib import ExitStack

import concourse.bass as bass
import concourse.tile as tile
from concourse import bacc, mybir
from concourse.bass_utils import run_bass_kernel_spmd

F32 = mybir.dt.float32
BF16 = mybir.dt.bfloat16
I32 = mybir.dt.int32
AF = mybir.ActivationFunctionType
ALU = mybir.AluOpType

B, T, H = 32, 1024, 1024
NCORES = 8
BPC = B // NCORES          # batches per core
RT = T // 128              # 8 row tiles
NTRI = RT * (RT + 1) // 2  # 36 lower-triangular tile pairs
NCH = H // 512             # 2 free-dim chunks
LN_EPS = 1e-5
QUAKE = 0x5F3759DF


def tri0(m):
    return m * (m + 1) // 2


def build(apply_g1=False, apply_b1=False, time_reps=1, bpc=BPC, **_):
    DT = BF16
    nc = bacc.Bacc("TRN2", target_bir_lowering=False, debug=False,
                   num_devices=NCORES)
    x_d = nc.dram_tensor("x", [bpc, 128, RT, H], DT, kind="ExternalInput").ap()
    w1_d = nc.dram_tensor("w1", [128, NTRI, 128], DT, kind="ExternalInput").ap()
    w2_d = nc.dram_tensor("w2", [128, NTRI, 128], DT, kind="ExternalInput").ap()
    cw1_d = nc.dram_tensor("cw1", [128, RT, RT, 128], DT, kind="ExternalInput").ap()
    cw2_d = nc.dram_tensor("cw2", [128, RT, H], DT, kind="ExternalInput").ap()
    tb1_d = nc.dram_tensor("tb1", [128, RT], F32, kind="ExternalInput").ap()
    tb2_d = nc.dram_tensor("tb2", [128, RT], F32, kind="ExternalInput").ap()
    cb1_d = nc.dram_tensor("cb1", [128, RT], F32, kind="ExternalInput").ap()
    cb2_d = nc.dram_tensor("cb2", [H], DT, kind="ExternalInput").ap()
    g1_d = nc.dram_tensor("g1", [H], F32, kind="ExternalInput").ap()
    b1_d = nc.dram_tensor("b1", [H], F32, kind="ExternalInput").ap()
    out_d = nc.dram_tensor("out", [bpc, 128, RT, H], F32, kind="ExternalOutput").ap()

    def bcast(ap_1d, n):
        return bass.AP(tensor=ap_1d.tensor, offset=ap_1d.offset,
                       ap=[[0, 128], [1, n]])

    with tile.TileContext(nc) as tc:
        with ExitStack() as ctx:
            singles = ctx.enter_context(tc.tile_pool(name="singles", bufs=1))
            xp = ctx.enter_context(tc.tile_pool(name="xp", bufs=3))
            pA = ctx.enter_context(tc.tile_pool(name="pA", bufs=2))
            pU = ctx.enter_context(tc.tile_pool(name="pU", bufs=1))
            pT = ctx.enter_context(tc.tile_pool(name="pT", bufs=2))
            prep = ctx.enter_context(tc.tile_pool(name="prep", bufs=2))
            scr = ctx.enter_context(tc.tile_pool(name="scr", bufs=2))
            otp = ctx.enter_context(tc.tile_pool(name="otp", bufs=3))
            stats = ctx.enter_context(tc.tile_pool(name="stats", bufs=4))
            psum = ctx.enter_context(tc.tile_pool(name="psum", bufs=8, space="PSUM"))

            # ---- resident weights + constants ----
            w1_sb = singles.tile([128, NTRI, 128], DT)
            w2_sb = singles.tile([128, NTRI, 128], DT)
            cw1_sb = singles.tile([128, RT, RT, 128], DT)
            cw2_sb = singles.tile([128, RT, H], DT)
            tb1_sb = singles.tile([128, RT], F32)
            tb2_sb = singles.tile([128, RT], F32)
            cb1_sb = singles.tile([128, RT], F32)
            cb2_sb = singles.tile([128, H], DT)
            # w1 in two pieces: the first 10 tri-tiles (m<4 chains) land
            # early so MM1(0)'s first matmuls aren't gated on the full load
            nc.sync.dma_start(w1_sb[:, 0:10, :], w1_d[:, 0:10, :])
            nc.sync.dma_start(w1_sb[:, 10:NTRI, :], w1_d[:, 10:NTRI, :])
            nc.sync.dma_start(w2_sb[:], w2_d[:])
            nc.sync.dma_start(cw1_sb[:], cw1_d[:])
            nc.sync.dma_start(cw2_sb[:], cw2_d[:])
            nc.sync.dma_start(tb1_sb[:], tb1_d[:])
            nc.sync.dma_start(tb2_sb[:], tb2_d[:])
            nc.sync.dma_start(cb1_sb[:], cb1_d[:])
            nc.sync.dma_start(cb2_sb[:], bcast(cb2_d, H))
            if apply_g1:
                g1_sb = singles.tile([128, H], F32)
                nc.sync.dma_start(g1_sb[:], bcast(g1_d, H))
            if apply_b1:
                b1_sb = singles.tile([128, H], F32)
                nc.sync.dma_start(b1_sb[:], bcast(b1_d, H))

            def row_stats(x_row, mv_out):
                """bn mean/var of one [128, H] row -> mv_out [128, 2]"""
                st = stats.tile([128, 2, 6], F32, name="st", tag="st")
                nc.vector.bn_stats(st[:, 0, :], x_row[:, 0:512])
                nc.vector.bn_stats(st[:, 1, :], x_row[:, 512:1024])
                nc.vector.bn_aggr(mv_out, st[:])

            def rsqrt_cols(mv_all, rs, m2, lo, hi, tag):
                """cols lo:hi: rs = 1/sqrt(var+eps), m2 = -mean*rs.
                quake initial guess + 2 Newton steps, vector engine only."""
                n = hi - lo
                ve = stats.tile([128, n], F32, name="ve", tag="ve" + tag)
                qt = stats.tile([128, n], F32, name="qt", tag="qt" + tag)
                rsv = rs[:, lo:hi]
                nc.vector.tensor_scalar_add(out=ve[:], in0=mv_all[:, lo:hi, 1],
                                            scalar1=LN_EPS)
                nc.vector.tensor_scalar(
                    out=rsv.bitcast(I32), in0=ve[:].bitcast(I32),
                    scalar1=1, scalar2=None, op0=ALU.arith_shift_right)
                nc.vector.tensor_scalar(
                    out=rsv.bitcast(I32), in0=rsv.bitcast(I32),
                    scalar1=-1, scalar2=QUAKE, op0=ALU.mult, op1=ALU.add)
                for _ in range(2):
                    nc.vector.tensor_mul(qt[:], rsv, rsv)
                    nc.vector.tensor_mul(qt[:], qt[:], ve[:])
                    nc.vector.tensor_scalar(out=qt[:], in0=qt[:], scalar1=-0.5,
                                            scalar2=1.5, op0=ALU.mult, op1=ALU.add)
                    nc.vector.tensor_mul(rsv, rsv, qt[:])
                nc.vector.scalar_tensor_tensor(
                    out=m2[:, lo:hi], in0=mv_all[:, lo:hi, 0], scalar=-1.0,
                    in1=rsv, op0=ALU.mult, op1=ALU.mult)

            def ln_apply(dst, src_row, rs, m2, r):
                """dst = src*rs + m2 (m2 = -mean*rs). bf16 in/out hits the
                vector engine's 2x mode, and keeping the whole LN chain
                (stats -> rsqrt -> apply) on one engine kills the
                cross-engine semaphore latency."""
                nc.vector.tensor_scalar(
                    out=dst, in0=src_row, scalar1=rs[:, r:r + 1],
                    scalar2=m2[:, r:r + 1], op0=ALU.mult, op1=ALU.add)
                if apply_g1:
                    nc.vector.tensor_mul(dst, dst, g1_sb[:])
                if apply_b1:
                    nc.vector.tensor_add(dst, dst, b1_sb[:])

            def load_x(b, chunks=4):
                """DMA one batch of x on the Activation queue."""
                x_sb = xp.tile([128, RT, H], DT, name="x_sb", tag="x")
                rows = RT // chunks
                for c in range(chunks):
                    nc.scalar.dma_start(
                        x_sb[:, rows * c:rows * (c + 1), :],
                        x_d[b][:, rows * c:rows * (c + 1), :])
                return x_sb

            def mm1(y_sb):
                """temporal MM1 + gelu -> u1g (bf16)"""
                u1g = pU.tile([128, RT, H], DT, tag="U")
                for m in range(RT):
                    pns = [psum.tile([128, 512], F32, name="pns", tag="ps")
                           for _ in range(NCH)]
                    for k in range(m + 1):
                        for n in range(NCH):
                            nc.tensor.matmul(
                                pns[n][:], w1_sb[:, tri0(m) + k, :],
                                y_sb[:, k, 512 * n:512 * n + 512],
                                start=(k == 0), stop=(k == m))
                    for n in range(NCH):
                        nc.scalar.activation(
                            u1g[:, m, 512 * n:512 * n + 512], pns[n][:],
                            AF.Gelu, bias=tb1_sb[:, m:m + 1])
                return u1g

            def ln1_prep(xn, fine=False):
                """LN1 stats + rsqrt + apply, all on the vector engine.
                fine=True emits rows 0/1 individually then the rest in one
                batch - minimizes the cold-start latency before MM1(0)."""
                mv1 = stats.tile([128, RT, 2], F32, name="mv1", tag="mv1")
                rs1 = stats.tile([128, RT], F32, name="rs1", tag="rs1")
                m21 = stats.tile([128, RT], F32, name="m21", tag="m21")
                yn = pA.tile([128, RT, H], DT, tag="A")
                if fine:
                    # rows in pairs: row r lands on the vector engine every
                    # ~2 bn_stats + one short rsqrt, keeping pace with MM1's
                    # triangular (cheap-early) chain progression
                    for r in range(RT):
                        row_stats(xn[:, r, :], mv1[:, r, :])
                        if r % 2 == 1:
                            rsqrt_cols(mv1, rs1, m21, r - 1, r + 1, "1")
                            ln_apply(yn[:, r - 1, :], xn[:, r - 1, :],
                                     rs1, m21, r - 1)
                            ln_apply(yn[:, r, :], xn[:, r, :], rs1, m21, r)
                else:
                    for r in range(RT):
                        row_stats(xn[:, r, :], mv1[:, r, :])
                    rsqrt_cols(mv1, rs1, m21, 0, RT, "1")
                    for r in range(RT):
                        ln_apply(yn[:, r, :], xn[:, r, :], rs1, m21, r)
                return yn

            # software-pipeline state
            st_x = [None] * bpc      # x tiles (trunk, becomes x2+cb2)
            st_y = [None]            # y of next batch
            st_u = [None]            # u1g of current batch
            st_t = [None] * bpc      # y2T tiles
            st_v = [None] * bpc      # v1g tiles

            def temporal_phase(b, bpc_n):
                """MM2(b) + LN2(b) + prep of batch b+1. Fills st_t[b]."""
                x_sb = st_x[b]
                u1g = st_u[0]
                nxt = b + 1 < bpc_n
                if nxt:
                    xn = load_x(b + 1)
                    st_x[b + 1] = xn

                mv2 = stats.tile([128, RT, 2], F32, name="mv2", tag="mv2")
                rs2 = stats.tile([128, RT], F32, name="rs2", tag="rs2")
                m22 = stats.tile([128, RT], F32, name="m22", tag="m22")
                acc2 = stats.tile([128, 2 * RT], F32, name="acc2", tag="acc2")
                sq2 = stats.tile([128, RT], F32, name="sq2", tag="sq2")

                # ---- temporal MM2 + bias + residual -> x_sb (bf16 trunk);
                #      LN2 sums fused; next batch's LN1 stats interleaved ----
                if nxt:
                    mv1n = stats.tile([128, RT, 2], F32, name="mv1", tag="mv1")
                    rs1n = stats.tile([128, RT], F32, name="rs1", tag="rs1")
                    m21n = stats.tile([128, RT], F32, name="m21", tag="m21")
                for m in range(RT):
                    pns = [psum.tile([128, 512], F32, name="pns", tag="ps")
                           for _ in range(NCH)]
                    for k in range(m + 1):
                        for n in range(NCH):
                            nc.tensor.matmul(
                                pns[n][:], w2_sb[:, tri0(m) + k, :],
                                u1g[:, k, 512 * n:512 * n + 512],
                                start=(k == 0), stop=(k == m))
                    # epilogues BEFORE next-batch stats: their PSUM reads
                    # unblock the ring slots chMM1(it-1) reuses 8 allocs later
                    for n in range(NCH):
                        sl = slice(512 * n, 512 * n + 512)
                        nc.vector.scalar_tensor_tensor(
                            out=x_sb[:, m, sl], in0=pns[n][:],
                            scalar=tb2_sb[:, m:m + 1], in1=x_sb[:, m, sl],
                            op0=ALU.add, op1=ALU.add,
                            accum_out=acc2[:, 2 * m + n:2 * m + n + 1])
                    if nxt:
                        row_stats(xn[:, m, :], mv1n[:, m, :])
                    sc = scr.tile([128, H], DT, name="sc", tag="sc")
                    nc.scalar.activation(sc[:], x_sb[:, m, :], AF.Square,
                                         accum_out=sq2[:, m:m + 1])

                # next batch's LN1 finish + applies (relaxed: consumer MM1
                # doesn't run until after chMM1 of the previous batch)
                if nxt:
                    rsqrt_cols(mv1n, rs1n, m21n, 0, RT, "1")
                    yn = pA.tile([128, RT, H], DT, tag="A")
                    for r in range(RT):
                        ln_apply(yn[:, r, :], xn[:, r, :], rs1n, m21n, r)
                    st_y[0] = yn

                # ---- LN2 moments from fused sums: mean = S/H,
                #      var = SQ/H - mean^2; apply + XBAR transpose ----
                nc.vector.tensor_tensor(
                    out=mv2[:, :, 0], in0=acc2[:, 0:2 * RT:2],
                    in1=acc2[:, 1:2 * RT:2], op=ALU.add)
                nc.vector.tensor_scalar_mul(out=mv2[:, :, 0], in0=mv2[:, :, 0],
                                            scalar1=1.0 / H)
                nc.vector.tensor_mul(mv2[:, :, 1], mv2[:, :, 0], mv2[:, :, 0])
                nc.vector.scalar_tensor_tensor(
                    out=mv2[:, :, 1], in0=sq2[:], scalar=1.0 / H,
                    in1=mv2[:, :, 1], op0=ALU.mult, op1=ALU.subtract)
                rsqrt_cols(mv2, rs2, m22, 0, RT, "2")
                # y2T[p, r, c, j] = y2[t = 128*r + j, h = 128*c + p]
                y2T = pT.tile([128, RT, RT, 128], DT, tag="T")
                for r in range(RT):
                    y2pre = prep.tile([128, H], DT, name="y2pre", tag="prep")
                    ln_apply(y2pre[:], x_sb[:, r, :], rs2, m22, r)
                    nc.sync.dma_start(y2T[:, r, :, :], y2pre[:], transpose=True)
                st_t[b] = y2T

                # fold cb2 into x2 rows (the store epilogue becomes 1 vec op);
                # bf16 adds on the vector engine - gpsimd would hold the
                # shared SBUF port pair and block concurrent DVE ops
                for r in range(RT):
                    nc.vector.tensor_add(x_sb[:, r, :], x_sb[:, r, :], cb2_sb[:])

            def ch_mm1(b):
                """channel MM1 + gelu -> v1g (in [H,T] layout)"""
                y2T = st_t[b]
                v1g = pA.tile([128, RT, H], DT, tag="A")
                for mo in range(RT):
                    pns = [psum.tile([128, 512], F32, name="pns", tag="ps")
                           for _ in range(NCH)]
                    for kh in range(RT):
                        for n in range(NCH):
                            nc.tensor.matmul(
                                pns[n][:], cw1_sb[:, kh, mo, :],
                                y2T[:, 4 * n:4 * n + 4, kh, :],
                                start=(kh == 0), stop=(kh == RT - 1))
                    for n in range(NCH):
                        nc.scalar.activation(
                            v1g[:, mo, 512 * n:512 * n + 512], pns[n][:],
                            AF.Gelu, bias=cb1_sb[:, mo:mo + 1])
                st_v[b] = v1g

            def ch_mm2(b):
                """channel MM2 (paired chains alternate PSUM banks)
                + residual (cb2 pre-folded into x2) -> out"""
                v1g = st_v[b]
                x_sb = st_x[b]
                for n2 in range(NCH):
                    sl = slice(512 * n2, 512 * n2 + 512)
                    for mt in range(0, RT, 2):
                        pp = [psum.tile([128, 512], F32, name="pns", tag="ps")
                              for _ in range(2)]
                        for ko in range(RT):
                            for i in range(2):
                                nc.tensor.matmul(
                                    pp[i][:],
                                    v1g[:, ko, 128 * (mt + i):128 * (mt + i) + 128],
                                    cw2_sb[:, ko, sl],
                                    start=(ko == 0), stop=(ko == RT - 1))
                        for i in range(2):
                            o_t = otp.tile([128, 512], F32, name="o_t", tag="o_t")
                            nc.vector.scalar_tensor_tensor(
                                out=o_t[:], in0=pp[i][:], scalar=1.0,
                                in1=x_sb[:, mt + i, sl],
                                op0=ALU.mult, op1=ALU.add)
                            nc.sync.dma_start(out_d[b][:, mt + i, sl], o_t[:])

            def run_all():
                st_x[0] = load_x(0, chunks=RT)
                st_y[0] = ln1_prep(st_x[0], fine=True)
                st_u[0] = mm1(st_y[0])
                # iteration it: PE order MM2(it), chMM1(it-1), MM1(it+1),
                # chMM2(it-1); channel work lags temporal by one iteration.
                for it in range(bpc + 1):
                    if it < bpc:
                        temporal_phase(it, bpc)
                    if it > 0:
                        ch_mm1(it - 1)
                    if it + 1 < bpc:
                        st_u[0] = mm1(st_y[0])
                    if it > 0:
                        ch_mm2(it - 1)

            if time_reps > 1:
                with tc.For_i(0, time_reps, 1,
                              hint_engines=(mybir.EngineType.PE,
                                            mybir.EngineType.DVE,
                                            mybir.EngineType.Activation,
                                            mybir.EngineType.SP,
                                            mybir.EngineType.Pool)):
                    run_all()
            else:
                run_all()

    nc.compile()
    return nc


def prep_inputs(x, tw1, tb1, tw2, tb2, cw1, cb1, cw2, cb2,
                ln1_g, ln1_b, ln2_g, ln2_b):
    """Host-side layout + weight folding. Returns (in_maps, apply_g1, apply_b1)."""
    f = np.float32
    bf = ml_dtypes.bfloat16
    x = np.ascontiguousarray(np.asarray(x, f))
    mask = np.tril(np.ones((T, T), f))
    w1mT = (mask * np.asarray(tw1, f)).T          # [j, i]
    w2mT = (mask * np.asarray(tw2, f)).T
    cw1 = np.asarray(cw1, f)
    cw2 = np.asarray(cw2, f)
    ln2_g = np.asarray(ln2_g, f)
    ln2_b = np.asarray(ln2_b, f)
    # fold LN2 affine into channel MLP first layer
    cw1p = cw1 * ln2_g[None, :]                   # [o, h]
    cb1p = np.asarray(cb1, f) + cw1 @ ln2_b       # [o]
    cw1pT = cw1p.T                                # [h, o]
    cw2T = cw2.T                                  # [o, p]

    def tiles4(w):   # [1024,1024] -> [128, 8, 8, 128] (p=row%128, k, m, col%128)
        return np.ascontiguousarray(
            w.reshape(RT, 128, RT, 128).transpose(1, 0, 2, 3))

    def tiles3(w):   # [1024,1024] -> [128, 8, 1024]
        return np.ascontiguousarray(w.reshape(RT, 128, H).transpose(1, 0, 2))

    def pack_tri(w4):  # [128, k, m, 128] -> [128, 36, 128], k<=m tiles only
        return np.ascontiguousarray(
            np.concatenate([w4[:, 0:m + 1, m, :] for m in range(RT)], axis=1))

    def bias_t(v):   # [1024] -> [128, 8]
        return np.ascontiguousarray(np.asarray(v, f).reshape(RT, 128).T)

    g1 = np.asarray(ln1_g, f)
    b1 = np.asarray(ln1_b, f)
    apply_g1 = not np.all(g1 == 1.0)
    apply_b1 = not np.all(b1 == 0.0)

    shared = {
        "w1": pack_tri(tiles4(w1mT)).astype(bf),
        "w2": pack_tri(tiles4(w2mT)).astype(bf),
        "cw1": tiles4(cw1pT).astype(bf),
        "cw2": tiles3(cw2T).astype(bf),
        "tb1": bias_t(tb1), "tb2": bias_t(tb2), "cb1": bias_t(cb1p),
        "cb2": np.ascontiguousarray(np.asarray(cb2, f)).astype(bf),
        "g1": np.ascontiguousarray(g1), "b1": np.ascontiguousarray(b1),
    }
    # x: [B,T,H] -> per-core [BPC, 128, RT, H]  (t = r*128 + p), bf16
    xs = x.reshape(NCORES, BPC, RT, 128, H).transpose(0, 1, 3, 2, 4)
    in_maps = [{"x": np.ascontiguousarray(xs[c]).astype(bf), **shared}
               for c in range(NCORES)]
    return in_maps, apply_g1, apply_b1


_cache = {}


def kernel(**inputs) -> np.ndarray:
    in_maps, apply_g1, apply_b1 = prep_inputs(**inputs)
    key = (apply_g1, apply_b1)
    if key not in _cache:
        _cache[key] = build(apply_g1=apply_g1, apply_b1=apply_b1, time_reps=1)
    nc = _cache[key]
    res = run_bass_kernel_spmd(nc, in_maps, list(range(NCORES)))
    # out per core: [BPC, 128, RT, H] -> [BPC, T, H]
    outs = [r["out"].transpose(0, 2, 1, 3).reshape(BPC, T, H)
            for r in res.results]
    return np.ascontiguousarray(np.concatenate(outs, axis=0), dtype=np.float32)


# revision 14
# speedup vs baseline: 1.0125x; 1.0125x over previous
"""MixerBlock kernel for Trainium2 (8 NeuronCores, data-parallel over batch). v4

Reference computation (per batch b of x[B,T,H], B=32, T=H=1024):
  y   = LN1(x)                                    # over H
  u1  = gelu(W1m @ y + tb1 x 1)    W1m = tril*tw1 # temporal mix in [T,H] layout
  x2  = x + W2m @ u1 + tb2 x 1     W2m = tril*tw2
  y2  = LN2(x2)
  v1  = gelu(cw1' @ y2^T + cb1' x 1)              # [H,T] layout, LN2 g/b folded
  out = x2 + (v1^T' @ cw2T) + 1 x cb2             # back in [T,H]

Design (vs the f32r baseline at 634 us):
- bf16 weights, activations AND residual trunk (PSUM accumulation stays f32;
  x is converted to bf16 on the host). Triangular temporal weight tiles are
  host-packed into [128, 36, 128] so ALL weights stay resident in SBUF
  (~6.25 MB loaded once, was 50 MB of per-batch DMA).
- Software pipeline, one stage deep: iteration `it` runs, back-to-back on
  the PE, MM2(it) -> chMM1(it-1) -> MM1(it+1) -> chMM2(it-1). Every
  LayerNorm chain (stats -> rsqrt -> apply -> transpose) therefore has a
  full channel phase (~55 us) of slack before its consumer - the PE never
  waits on the vector/scalar engines.
- The y2 transpose runs on the DMA engines' XBAR (dma_start(transpose=True),
  out[p,c,j] = in[j,128c+p]) instead of 256 PE transpose matmuls.
- Consecutive matmuls always alternate PSUM banks (incl. chMM2, which pairs
  two output chains) - same-bank back-to-back matmuls serialize fill/drain
  (~405 ns vs ~216 ns per N=512 matmul, HW-measured).
- LayerNorm applies run on the scalar engine as Identity activations
  (out = in*rs + (-mean*rs)); identity/square/gelu share one act table so
  the scalar engine never reloads tables (~2.7 us each). LN2 row sums ride
  the MM2 epilogue's accum_out; sumsq comes from a Square pass.
- 1/sqrt(var+eps) via quake-rsqrt + 2 Newton steps on the vector engine.
- gpsimd only does the cb2-fold (gpsimd ops hold the shared DVE/GpSimd SBUF
  port pair for their whole duration, blocking concurrent DVE ops).
- x loads use the Activation HWDGE queue; stores/transposes use the SP
  queue, so batch-(b+1) loads never queue behind batch-b stores.
"""
import os
import numpy as np
import ml_dtypes
from contextlib import ExitStack

import concourse.bass as bass
import concourse.tile as tile
from concourse import bacc, mybir
from concourse.bass_utils import run_bass_kernel_spmd

F32 = mybir.dt.float32
BF16 = mybir.dt.bfloat16
I32 = mybir.dt.int32
AF = mybir.ActivationFunctionType
ALU = mybir.AluOpType

B, T, H = 32, 1024, 1024
NCORES = 8
BPC = B // NCORES          # batches per core
RT = T // 128              # 8 row tiles
NTRI = RT * (RT + 1) // 2  # 36 lower-triangular tile pairs
NCH = H // 512             # 2 free-dim chunks
LN_EPS = 1e-5
QUAKE = 0x5F3759DF


def tri0(m):
    return m * (m + 1) // 2


def build(apply_g1=False, apply_b1=False, time_reps=1, bpc=BPC, **_):
    DT = BF16
    nc = bacc.Bacc("TRN2", target_bir_lowering=False, debug=False,
                   num_devices=NCORES)
    x_d = nc.dram_tensor("x", [bpc, 128, RT, H], DT, kind="ExternalInput").ap()
    w1_d = nc.dram_tensor("w1", [128, NTRI, 128], DT, kind="ExternalInput").ap()
    w2_d = nc.dram_tensor("w2", [128, NTRI, 128], DT, kind="ExternalInput").ap()
    cw1_d = nc.dram_tensor("cw1", [128, RT, RT, 128], DT, kind="ExternalInput").ap()
    cw2_d = nc.dram_tensor("cw2", [128, RT, H], DT, kind="ExternalInput").ap()
    tb1_d = nc.dram_tensor("tb1", [128, RT], F32, kind="ExternalInput").ap()
    tb2_d = nc.dram_tensor("tb2", [128, RT], F32, kind="ExternalInput").ap()
    cb1_d = nc.dram_tensor("cb1", [128, RT], F32, kind="ExternalInput").ap()
    cb2_d = nc.dram_tensor("cb2", [H], DT, kind="ExternalInput").ap()
    g1_d = nc.dram_tensor("g1", [H], F32, kind="ExternalInput").ap()
    b1_d = nc.dram_tensor("b1", [H], F32, kind="ExternalInput").ap()
    out_d = nc.dram_tensor("out", [bpc, 128, RT, H], F32, kind="ExternalOutput").ap()

    def bcast(ap_1d, n):
        return bass.AP(tensor=ap_1d.tensor, offset=ap_1d.offset,
                       ap=[[0, 128], [1, n]])

    with tile.TileContext(nc) as tc:
        with ExitStack() as ctx:
            singles = ctx.enter_context(tc.tile_pool(name="singles", bufs=1))
            xp = ctx.enter_context(tc.tile_pool(name="xp", bufs=3))
            pA = ctx.enter_context(tc.tile_pool(name="pA", bufs=2))
            pU = ctx.enter_context(tc.tile_pool(name="pU", bufs=1))
            pT = ctx.enter_context(tc.tile_pool(name="pT", bufs=2))
            prep = ctx.enter_context(tc.tile_pool(name="prep", bufs=2))
            scr = ctx.enter_context(tc.tile_pool(name="scr", bufs=2))
            otp = ctx.enter_context(tc.tile_pool(name="otp", bufs=3))
            stats = ctx.enter_context(tc.tile_pool(name="stats", bufs=4))
            psum = ctx.enter_context(tc.tile_pool(name="psum", bufs=8, space="PSUM"))

            # ---- resident weights + constants ----
            w1_sb = singles.tile([128, NTRI, 128], DT)
            w2_sb = singles.tile([128, NTRI, 128], DT)
            cw1_sb = singles.tile([128, RT, RT, 128], DT)
            cw2_sb = singles.tile([128, RT, H], DT)
            tb1_sb = singles.tile([128, RT], F32)
            tb2_sb = singles.tile([128, RT], F32)
            cb1_sb = singles.tile([128, RT], F32)
            cb2_sb = singles.tile([128, H], DT)
            nc.sync.dma_start(w1_sb[:], w1_d[:])
            nc.sync.dma_start(w2_sb[:], w2_d[:])
            nc.sync.dma_start(cw1_sb[:], cw1_d[:])
            nc.sync.dma_start(cw2_sb[:], cw2_d[:])
            nc.sync.dma_start(tb1_sb[:], tb1_d[:])
            nc.sync.dma_start(tb2_sb[:], tb2_d[:])
            nc.sync.dma_start(cb1_sb[:], cb1_d[:])
            nc.sync.dma_start(cb2_sb[:], bcast(cb2_d, H))
            if apply_g1:
                g1_sb = singles.tile([128, H], F32)
                nc.sync.dma_start(g1_sb[:], bcast(g1_d, H))
            if apply_b1:
                b1_sb = singles.tile([128, H], F32)
                nc.sync.dma_start(b1_sb[:], bcast(b1_d, H))

            def row_stats(x_row, mv_out):
                """bn mean/var of one [128, H] row -> mv_out [128, 2]"""
                st = stats.tile([128, 2, 6], F32, name="st", tag="st")
                nc.vector.bn_stats(st[:, 0, :], x_row[:, 0:512])
                nc.vector.bn_stats(st[:, 1, :], x_row[:, 512:1024])
                nc.vector.bn_aggr(mv_out, st[:])

            def rsqrt_cols(mv_all, rs, m2, lo, hi, tag):
                """cols lo:hi: rs = 1/sqrt(var+eps), m2 = -mean*rs.
                quake initial guess + 2 Newton steps, vector engine only."""
                n = hi - lo
                ve = stats.tile([128, n], F32, name="ve", tag="ve" + tag)
                qt = stats.tile([128, n], F32, name="qt", tag="qt" + tag)
                rsv = rs[:, lo:hi]
                nc.vector.tensor_scalar_add(out=ve[:], in0=mv_all[:, lo:hi, 1],
                                            scalar1=LN_EPS)
                nc.vector.tensor_scalar(
                    out=rsv.bitcast(I32), in0=ve[:].bitcast(I32),
                    scalar1=1, scalar2=None, op0=ALU.arith_shift_right)
                nc.vector.tensor_scalar(
                    out=rsv.bitcast(I32), in0=rsv.bitcast(I32),
                    scalar1=-1, scalar2=QUAKE, op0=ALU.mult, op1=ALU.add)
                for _ in range(2):
                    nc.vector.tensor_mul(qt[:], rsv, rsv)
                    nc.vector.tensor_mul(qt[:], qt[:], ve[:])
                    nc.vector.tensor_scalar(out=qt[:], in0=qt[:], scalar1=-0.5,
                                            scalar2=1.5, op0=ALU.mult, op1=ALU.add)
                    nc.vector.tensor_mul(rsv, rsv, qt[:])
                nc.vector.scalar_tensor_tensor(
                    out=m2[:, lo:hi], in0=mv_all[:, lo:hi, 0], scalar=-1.0,
                    in1=rsv, op0=ALU.mult, op1=ALU.mult)

            def ln_apply(dst, src_row, rs, m2, r):
                """dst = src*rs + m2 (m2 = -mean*rs). bf16 in/out hits the
                vector engine's 2x mode, and keeping the whole LN chain
                (stats -> rsqrt -> apply) on one engine kills the
                cross-engine semaphore latency."""
                nc.vector.tensor_scalar(
                    out=dst, in0=src_row, scalar1=rs[:, r:r + 1],
                    scalar2=m2[:, r:r + 1], op0=ALU.mult, op1=ALU.add)
                if apply_g1:
                    nc.vector.tensor_mul(dst, dst, g1_sb[:])
                if apply_b1:
                    nc.vector.tensor_add(dst, dst, b1_sb[:])

            def load_x(b, chunks=4):
                """DMA one batch of x on the Activation queue."""
                x_sb = xp.tile([128, RT, H], DT, name="x_sb", tag="x")
                rows = RT // chunks
                for c in range(chunks):
                    nc.scalar.dma_start(
                        x_sb[:, rows * c:rows * (c + 1), :],
                        x_d[b][:, rows * c:rows * (c + 1), :])
                return x_sb

            def mm1(y_sb):
                """temporal MM1 + gelu -> u1g (bf16)"""
                u1g = pU.tile([128, RT, H], DT, tag="U")
                for m in range(RT):
                    pns = [psum.tile([128, 512], F32, name="pns", tag="ps")
                           for _ in range(NCH)]
                    for k in range(m + 1):
                        for n in range(NCH):
                            nc.tensor.matmul(
                                pns[n][:], w1_sb[:, tri0(m) + k, :],
                                y_sb[:, k, 512 * n:512 * n + 512],
                                start=(k == 0), stop=(k == m))
                    for n in range(NCH):
                        nc.scalar.activation(
                            u1g[:, m, 512 * n:512 * n + 512], pns[n][:],
                            AF.Gelu, bias=tb1_sb[:, m:m + 1])
                return u1g

            def ln1_prep(xn, fine=False):
                """LN1 stats + rsqrt + apply, all on the vector engine.
                fine=True emits rows 0/1 individually then the rest in one
                batch - minimizes the cold-start latency before MM1(0)."""
                mv1 = stats.tile([128, RT, 2], F32, name="mv1", tag="mv1")
                rs1 = stats.tile([128, RT], F32, name="rs1", tag="rs1")
                m21 = stats.tile([128, RT], F32, name="m21", tag="m21")
                yn = pA.tile([128, RT, H], DT, tag="A")
                if fine:
                    # rows in pairs: row r lands on the vector engine every
                    # ~2 bn_stats + one short rsqrt, keeping pace with MM1's
                    # triangular (cheap-early) chain progression
                    for r in range(RT):
                        row_stats(xn[:, r, :], mv1[:, r, :])
                        if r % 2 == 1:
                            rsqrt_cols(mv1, rs1, m21, r - 1, r + 1, "1")
                            ln_apply(yn[:, r - 1, :], xn[:, r - 1, :],
                                     rs1, m21, r - 1)
                            ln_apply(yn[:, r, :], xn[:, r, :], rs1, m21, r)
                else:
                    for r in range(RT):
                        row_stats(xn[:, r, :], mv1[:, r, :])
                    rsqrt_cols(mv1, rs1, m21, 0, RT, "1")
                    for r in range(RT):
                        ln_apply(yn[:, r, :], xn[:, r, :], rs1, m21, r)
                return yn

            # software-pipeline state
            st_x = [None] * bpc      # x tiles (trunk, becomes x2+cb2)
            st_y = [None]            # y of next batch
            st_u = [None]            # u1g of current batch
            st_t = [None] * bpc      # y2T tiles
            st_v = [None] * bpc      # v1g tiles

            def temporal_phase(b, bpc_n):
                """MM2(b) + LN2(b) + prep of batch b+1. Fills st_t[b]."""
                x_sb = st_x[b]
                u1g = st_u[0]
                nxt = b + 1 < bpc_n
                if nxt:
                    xn = load_x(b + 1)
                    st_x[b + 1] = xn

                mv2 = stats.tile([128, RT, 2], F32, name="mv2", tag="mv2")
                rs2 = stats.tile([128, RT], F32, name="rs2", tag="rs2")
                m22 = stats.tile([128, RT], F32, name="m22", tag="m22")
                acc2 = stats.tile([128, 2 * RT], F32, name="acc2", tag="acc2")
                sq2 = stats.tile([128, RT], F32, name="sq2", tag="sq2")

                # ---- temporal MM2 + bias + residual -> x_sb (bf16 trunk);
                #      LN2 sums fused; next batch's LN1 stats interleaved ----
                if nxt:
                    mv1n = stats.tile([128, RT, 2], F32, name="mv1", tag="mv1")
                    rs1n = stats.tile([128, RT], F32, name="rs1", tag="rs1")
                    m21n = stats.tile([128, RT], F32, name="m21", tag="m21")
                for m in range(RT):
                    pns = [psum.tile([128, 512], F32, name="pns", tag="ps")
                           for _ in range(NCH)]
                    for k in range(m + 1):
                        for n in range(NCH):
                            nc.tensor.matmul(
                                pns[n][:], w2_sb[:, tri0(m) + k, :],
                                u1g[:, k, 512 * n:512 * n + 512],
                                start=(k == 0), stop=(k == m))
                    if nxt:
                        row_stats(xn[:, m, :], mv1n[:, m, :])
                    for n in range(NCH):
                        sl = slice(512 * n, 512 * n + 512)
                        nc.vector.scalar_tensor_tensor(
                            out=x_sb[:, m, sl], in0=pns[n][:],
                            scalar=tb2_sb[:, m:m + 1], in1=x_sb[:, m, sl],
                            op0=ALU.add, op1=ALU.add,
                            accum_out=acc2[:, 2 * m + n:2 * m + n + 1])
                    sc = scr.tile([128, H], DT, name="sc", tag="sc")
                    nc.scalar.activation(sc[:], x_sb[:, m, :], AF.Square,
                                         accum_out=sq2[:, m:m + 1])

                # next batch's LN1 finish + applies (relaxed: consumer MM1
                # doesn't run until after chMM1 of the previous batch)
                if nxt:
                    rsqrt_cols(mv1n, rs1n, m21n, 0, RT, "1")
                    yn = pA.tile([128, RT, H], DT, tag="A")
                    for r in range(RT):
                        ln_apply(yn[:, r, :], xn[:, r, :], rs1n, m21n, r)
                    st_y[0] = yn

                # ---- LN2 moments from fused sums: mean = S/H,
                #      var = SQ/H - mean^2; apply + XBAR transpose ----
                nc.vector.tensor_tensor(
                    out=mv2[:, :, 0], in0=acc2[:, 0:2 * RT:2],
                    in1=acc2[:, 1:2 * RT:2], op=ALU.add)
                nc.vector.tensor_scalar_mul(out=mv2[:, :, 0], in0=mv2[:, :, 0],
                                            scalar1=1.0 / H)
                nc.vector.tensor_mul(mv2[:, :, 1], mv2[:, :, 0], mv2[:, :, 0])
                nc.vector.scalar_tensor_tensor(
                    out=mv2[:, :, 1], in0=sq2[:], scalar=1.0 / H,
                    in1=mv2[:, :, 1], op0=ALU.mult, op1=ALU.subtract)
                rsqrt_cols(mv2, rs2, m22, 0, RT, "2")
                # y2T[p, r, c, j] = y2[t = 128*r + j, h = 128*c + p]
                y2T = pT.tile([128, RT, RT, 128], DT, tag="T")
                for r in range(RT):
                    y2pre = prep.tile([128, H], DT, name="y2pre", tag="prep")
                    ln_apply(y2pre[:], x_sb[:, r, :], rs2, m22, r)
                    nc.sync.dma_start(y2T[:, r, :, :], y2pre[:], transpose=True)
                st_t[b] = y2T

                # fold cb2 into x2 rows (the store epilogue becomes 1 vec op);
                # bf16 adds on the vector engine - gpsimd would hold the
                # shared SBUF port pair and block concurrent DVE ops
                for r in range(RT):
                    nc.vector.tensor_add(x_sb[:, r, :], x_sb[:, r, :], cb2_sb[:])

            def ch_mm1(b):
                """channel MM1 + gelu -> v1g (in [H,T] layout)"""
                y2T = st_t[b]
                v1g = pA.tile([128, RT, H], DT, tag="A")
                for mo in range(RT):
                    pns = [psum.tile([128, 512], F32, name="pns", tag="ps")
                           for _ in range(NCH)]
                    for kh in range(RT):
                        for n in range(NCH):
                            nc.tensor.matmul(
                                pns[n][:], cw1_sb[:, kh, mo, :],
                                y2T[:, 4 * n:4 * n + 4, kh, :],
                                start=(kh == 0), stop=(kh == RT - 1))
                    for n in range(NCH):
                        nc.scalar.activation(
                            v1g[:, mo, 512 * n:512 * n + 512], pns[n][:],
                            AF.Gelu, bias=cb1_sb[:, mo:mo + 1])
                st_v[b] = v1g

            def ch_mm2(b):
                """channel MM2 (paired chains alternate PSUM banks)
                + residual (cb2 pre-folded into x2) -> out"""
                v1g = st_v[b]
                x_sb = st_x[b]
                for n2 in range(NCH):
                    sl = slice(512 * n2, 512 * n2 + 512)
                    for mt in range(0, RT, 2):
                        pp = [psum.tile([128, 512], F32, name="pns", tag="ps")
                              for _ in range(2)]
                        for ko in range(RT):
                            for i in range(2):
                                nc.tensor.matmul(
                                    pp[i][:],
                                    v1g[:, ko, 128 * (mt + i):128 * (mt + i) + 128],
                                    cw2_sb[:, ko, sl],
                                    start=(ko == 0), stop=(ko == RT - 1))
                        for i in range(2):
                            o_t = otp.tile([128, 512], F32, name="o_t", tag="o_t")
                            nc.vector.scalar_tensor_tensor(
                                out=o_t[:], in0=pp[i][:], scalar=1.0,
                                in1=x_sb[:, mt + i, sl],
                                op0=ALU.mult, op1=ALU.add)
                            nc.sync.dma_start(out_d[b][:, mt + i, sl], o_t[:])

            def run_all():
                st_x[0] = load_x(0, chunks=RT)
                st_y[0] = ln1_prep(st_x[0], fine=True)
                st_u[0] = mm1(st_y[0])
                # iteration it: PE order MM2(it), chMM1(it-1), MM1(it+1),
                # chMM2(it-1); channel work lags temporal by one iteration.
                for it in range(bpc + 1):
                    if it < bpc:
                        temporal_phase(it, bpc)
                    if it > 0:
                        ch_mm1(it - 1)
                    if it + 1 < bpc:
                        st_u[0] = mm1(st_y[0])
                    if it > 0:
                        ch_mm2(it - 1)

            if time_reps > 1:
                with tc.For_i(0, time_reps, 1,
                              hint_engines=(mybir.EngineType.PE,
                                            mybir.EngineType.DVE,
                                            mybir.EngineType.Activation,
                                            mybir.EngineType.SP,
                                            mybir.EngineType.Pool)):
                    run_all()
            else:
                run_all()

    nc.compile()
    return nc


def prep_inputs(x, tw1, tb1, tw2, tb2, cw1, cb1, cw2, cb2,
                ln1_g, ln1_b, ln2_g, ln2_b):
    """Host-side layout + weight folding. Returns (in_maps, apply_g1, apply_b1)."""
    f = np.float32
    bf = ml_dtypes.bfloat16
    x = np.ascontiguousarray(np.asarray(x, f))
    mask = np.tril(np.ones((T, T), f))
    w1mT = (mask * np.asarray(tw1, f)).T          # [j, i]
    w2mT = (mask * np.asarray(tw2, f)).T
    cw1 = np.asarray(cw1, f)
    cw2 = np.asarray(cw2, f)
    ln2_g = np.asarray(ln2_g, f)
    ln2_b = np.asarray(ln2_b, f)
    # fold LN2 affine into channel MLP first layer
    cw1p = cw1 * ln2_g[None, :]                   # [o, h]
    cb1p = np.asarray(cb1, f) + cw1 @ ln2_b       # [o]
    cw1pT = cw1p.T                                # [h, o]
    cw2T = cw2.T                                  # [o, p]

    def tiles4(w):   # [1024,1024] -> [128, 8, 8, 128] (p=row%128, k, m, col%128)
        return np.ascontiguousarray(
            w.reshape(RT, 128, RT, 128).transpose(1, 0, 2, 3))

    def tiles3(w):   # [1024,1024] -> [128, 8, 1024]
        return np.ascontiguousarray(w.reshape(RT, 128, H).transpose(1, 0, 2))

    def pack_tri(w4):  # [128, k, m, 128] -> [128, 36, 128], k<=m tiles only
        return np.ascontiguousarray(
            np.concatenate([w4[:, 0:m + 1, m, :] for m in range(RT)], axis=1))

    def bias_t(v):   # [1024] -> [128, 8]
        return np.ascontiguousarray(np.asarray(v, f).reshape(RT, 128).T)

    g1 = np.asarray(ln1_g, f)
    b1 = np.asarray(ln1_b, f)
    apply_g1 = not np.all(g1 == 1.0)
    apply_b1 = not np.all(b1 == 0.0)

    shared = {
        "w1": pack_tri(tiles4(w1mT)).astype(bf),
        "w2": pack_tri(tiles4(w2mT)).astype(bf),
        "cw1": tiles4(cw1pT).astype(bf),
        "cw2": tiles3(cw2T).astype(bf),
        "tb1": bias_t(tb1), "tb2": bias_t(tb2), "cb1": bias_t(cb1p),
        "cb2": np.ascontiguousarray(np.asarray(cb2, f)).astype(bf),
        "g1": np.ascontiguousarray(g1), "b1": np.ascontiguousarray(b1),
    }
    # x: [B,T,H] -> per-core [BPC, 128, RT, H]  (t = r*128 + p), bf16
    xs = x.reshape(NCORES, BPC, RT, 128, H).transpose(0, 1, 3, 2, 4)
    in_maps = [{"x": np.ascontiguousarray(xs[c]).astype(bf), **shared}
               for c in range(NCORES)]
    return in_maps, apply_g1, apply_b1


_cache = {}


def kernel(**inputs) -> np.ndarray:
    in_maps, apply_g1, apply_b1 = prep_inputs(**inputs)
    key = (apply_g1, apply_b1)
    if key not in _cache:
        _cache[key] = build(apply_g1=apply_g1, apply_b1=apply_b1, time_reps=1)
    nc = _cache[key]
    res = run_bass_kernel_spmd(nc, in_maps, list(range(NCORES)))
    # out per core: [BPC, 128, RT, H] -> [BPC, T, H]
    outs = [r["out"].transpose(0, 2, 1, 3).reshape(BPC, T, H)
            for r in res.results]
    return np.ascontiguousarray(np.concatenate(outs, axis=0), dtype=np.float32)
